# revision 8
# baseline (speedup 1.0000x reference)
"""Trainium2 Bass kernel for the DependencyAnalyzer GNN problem.

Computation (reference semantics):
    h = relu(features @ W_node + b_node)                  # [N, H]
    2x: agg = scatter_add(h[src] -> dst);  h = relu((h + agg) @ W_conv + b_conv)
    out = stack([ (m*h) @ (m*h).T,  h @ h.T ])            # m = (nodes == 2)

Strategy (8 NeuronCores, SPMD):
  - Host reformats the edge list into per-core dense adjacency blocks
    A'^T [src=8192, dst_local=1024] in fp8 (counts <= 16 are exact), with
    the identity folded in (A' = A + I); A' stays fp8 end-to-end as the
    *moving* matmul operand (full rate against an fp16 stationary side).
  - h0 (replicated) is software-pipelined into the round-1 n=0
    aggregation loop so the first AllGather input is ready right behind
    the A-load stream.
  - AllGathers carry fp16 hi-only h (64-128 KB): the collective runs at
    the fold_n bandwidth ceiling, so payload is wall-clock.
  - similarity row-slabs exploit symmetry: each 512-row group writes
    only col-groups at cyclic distance 0..8; the host reconstructs the
    rest by transposing mirror blocks.  The needed per-core rotation of
    the gathered h2 (local col-slot s <- rank (c+s)%8) is done with
    indirect DMA gathers driven by a host-provided index table — the
    only per-core-variant addressing in an otherwise shared program.
  - On-device output is fp16 scaled by 1/4 (exact rescale on host);
    function_deps is a row/col-masked copy of similarity applied on the
    host during unsharding.
"""

import numpy as np
import ml_dtypes

import concourse.bass as bass
import concourse.mybir as mybir
import concourse.tile as tile
from concourse import masks
from concourse.bass_utils import run_bass_kernel_spmd

N = 8192          # nodes
NB = 1024         # nodes per core block
NCORES = 8
F = 10            # feature dim
FA = F + 1        # +1 ones row (bias fold)
H = 64            # hidden dim
KT = N // 128     # 64 src k-tiles
MT = NB // 128    # 8 own m-tiles
AC = 8            # A-load chunks (8 k-tiles each)
NW = 9            # owned col-512-tiles per 512-row group (cyclic window)
F32 = mybir.dt.float32
BF16 = mybir.dt.bfloat16
F16 = mybir.dt.float16
F8 = mybir.dt.float8e4
I32 = mybir.dt.int32
RELU = mybir.ActivationFunctionType.Relu

LAST_RESULT = None  # BassKernelResults of the most recent run (for test harness)


def _ensure_trace_hook():
    """Best-effort: register the NTFF profiling hook for trace=True runs."""
    import sys as _sys
    import types as _types

    try:
        if "antenv.axon_hooks" in _sys.modules:
            return
        import antenv as _antenv

        mod = _types.ModuleType("antenv.axon_hooks")
        _state = {"hook": None}
        mod.set_axon_ntff_profile_hook = lambda h: _state.__setitem__("hook", h)
        mod.get_axon_ntff_profile_hook = lambda: _state["hook"]
        _sys.modules["antenv.axon_hooks"] = mod
        _antenv.axon_hooks = mod

        from trn_agent_boot.trn_boot import _ntff_profile_via_ctypes

        so_path = "/opt/axon/libaxon_pjrt.so"
        import os as _os

        if _os.path.exists(so_path):
            hook = _ntff_profile_via_ctypes(so_path)
            if hook is not None:
                mod.set_axon_ntff_profile_hook(hook)
    except Exception:
        pass


def _legalize_waits(nc, max_waits=1):
    """This walrus build accepts at most one sync-wait per lowered HW
    instruction; hoist extra waits onto standalone EventSemaphore
    instructions on the same (in-order) engine queue."""
    n_fixed = 0
    for f in nc.m.functions:
        for bb in f.blocks:
            new_list = []
            for ins in bb.instructions:
                si = ins.sync_info
                if si is not None and len(si.on_wait) > max_waits:
                    waits = list(si.on_wait)
                    for w in waits[: len(waits) - max_waits]:
                        ev = mybir.InstEventSemaphore(
                            name=f"{ins.name}-w-{w.ant_name}",
                            ins=[],
                            outs=[],
                            sync_info=mybir.SyncInfo(on_wait=[w], on_update=[]),
                            engine=ins.engine,
                        )
                        new_list.append(ev)
                    ins.sync_info = mybir.SyncInfo(
                        on_wait=waits[len(waits) - max_waits :],
                        on_update=list(si.on_update),
                    )
                    n_fixed += 1
                new_list.append(ins)
            bb.instructions = new_list
    return n_fixed


def _build_nc():
    nc = bass.Bass(num_devices=NCORES)

    # ---- external I/O (same program on all cores; per-core data differs) ----
    featT = nc.declare_dram_parameter("featT3", [3 * FA, N], BF16, isOutput=False)
    WnA = nc.declare_dram_parameter("W3", [3 * FA, H], BF16, isOutput=False)
    Wc2h = nc.declare_dram_parameter("Wc2h", [2 * H, H], F16, isOutput=False)
    Wc2l = nc.declare_dram_parameter("Wc2l", [2 * H, H], F16, isOutput=False)
    Wc1h = nc.declare_dram_parameter("Wc1h", [H, H], F16, isOutput=False)
    Wc1l = nc.declare_dram_parameter("Wc1l", [H, H], F16, isOutput=False)
    bc = nc.declare_dram_parameter("bc", [H, 1], F32, isOutput=False)
    bch = nc.declare_dram_parameter("bch", [H, 1], F32, isOutput=False)
    # rotation index table: ridx[p, s] = 64*((c+s)%8) + p%64
    ridx = nc.declare_dram_parameter("ridx", [128, NCORES], I32, isOutput=False)
    # A'^T src-tile-packed: AT[p, k*NB + d] = count for src (k//2)*256 +
    # (k%2)*128 + p, local dst d.
    AT = nc.declare_dram_parameter("AT", [128, KT * NB], F8, isOutput=False)
    out_ext = nc.declare_dram_parameter("out", [NB, (NW + 1) * 512], F16,
                                        isOutput=True)

    # ---- internal DRAM (collective bounce buffers) ----
    ag1a_in = nc.dram_tensor("ag1a_in", [NB // 2, H], F16)
    ag1a_out = nc.dram_tensor("ag1a_out", [N // 2, H], F16, addr_space="Shared")
    ag1b_in = nc.dram_tensor("ag1b_in", [NB // 2, H], F16)
    ag1b_out = nc.dram_tensor("ag1b_out", [N // 2, H], F16, addr_space="Shared")
    ag2_in = nc.dram_tensor("ag2_in", [H, NB], F16)
    ag2_out = nc.dram_tensor("ag2_out", [NCORES * H, NB], F16, addr_space="Shared")
    rg = [list(range(NCORES))]

    with tile.TileContext(nc, num_cores=NCORES) as tc:
        with tc.tile_pool(name="persist", bufs=1) as persist:
            # ---------------- constants / small inputs -----------------------
            # wn/bc on ACT HWDGE (needed early); other consts via SWDGE;
            # ft + A stream + outputs on the SP HWDGE queue.
            wn_s = persist.tile([3 * FA, H], BF16)
            nc.scalar.dma_start(out=wn_s[:], in_=WnA[:])
            bc_s = persist.tile([H, 1], F32)
            nc.scalar.dma_start(out=bc_s[:], in_=bc[:])
            bch_s = persist.tile([H, 1], F32)
            nc.scalar.dma_start(out=bch_s[:], in_=bch[:])
            ridx_s = persist.tile([128, NCORES], I32)
            nc.scalar.dma_start(out=ridx_s[:], in_=ridx[:])
            wc2h_s = persist.tile([2 * H, H], F16)
            nc.gpsimd.dma_start(out=wc2h_s[:], in_=Wc2h[:])
            wc2l_s = persist.tile([2 * H, H], F16)
            nc.gpsimd.dma_start(out=wc2l_s[:], in_=Wc2l[:])
            wc1h_s = persist.tile([H, H], F16)
            nc.gpsimd.dma_start(out=wc1h_s[:], in_=Wc1h[:])
            wc1l_s = persist.tile([H, H], F16)
            nc.gpsimd.dma_start(out=wc1l_s[:], in_=Wc1l[:])
            ident = persist.tile([H, H], F16)
            masks.make_identity(nc, ident[:])
            dummy_s = persist.tile([1, 512], BF16)
            nc.vector.memset(dummy_s[:], 0.0)

            def absorb(pt, parts, free):
                # Dummy full-tile matmul: soaks up PSUM pool-boundary WAR
                # waits on PE so real matmuls stay within the ISA's sync
                # wait budget.
                nc.tensor.matmul(
                    pt[:, :],
                    dummy_s[0:1, 0:parts],
                    dummy_s[0:1, 0:free],
                    start=True,
                    stop=True,
                )

            def warmers(n_fill, fill_ps):
                # Dependency-free matmuls into an already-drained psum bank;
                # they run while the PE would otherwise idle in a collective
                # wait, keeping the HAM clock gate at full rate.
                for _ in range(n_fill):
                    nc.tensor.matmul(
                        fill_ps[0:1, :], dummy_s[0:1, 0:1], dummy_s[0:1, :],
                        start=True, stop=True,
                    )

            # final-h (hi/lo fp16, T layout, x0.5) for the output matmuls
            hThl = persist.tile([128, NB], F16)

            with (
                tc.tile_pool(name="apool", bufs=AC) as apool,
                tc.tile_pool(name="hilo", bufs=KT) as hilopool,
                tc.tile_pool(name="ftp", bufs=2) as ftp,
            ):
                ft_halves = []
                for half in range(2):
                    ft_h = ftp.tile([3 * FA, N // 2], BF16, tag=f"ft{half}", bufs=1)
                    nc.sync.dma_start(
                        out=ft_h[:],
                        in_=featT[:, half * (N // 2) : (half + 1) * (N // 2)],
                    )
                    ft_halves.append(ft_h)

                a_chunks = []
                for j in range(AC):
                    at = apool.tile([128, (KT // AC) * NB], F8, name=f"a{j}",
                                    tag="A")
                    nc.sync.dma_start(
                        out=at[:],
                        in_=AT[:, j * (KT // AC) * NB : (j + 1) * (KT // AC) * NB],
                    )
                    a_chunks.append(at)

                def a_slice(k, n):
                    at = a_chunks[k // (KT // AC)]
                    off = (k % (KT // AC)) * NB + n * 512
                    return at[:, off : off + 512]

                # ---- phase 1 + round-1 n=0, software-pipelined -------------
                with tc.tile_pool(name="prd1", bufs=1, space="PSUM") as prd:
                    psa0 = prd.tile([128, 512], F32, tag="psa", bufs=2)
                    psa1 = prd.tile([128, 512], F32, tag="psa", bufs=2)
                    h0_tiles = []
                    LAG = 3
                    with tc.tile_pool(name="pp1", bufs=2, space="PSUM") as pp1:
                        for kk in range(KT + LAG):
                            if kk < KT:
                                k = kk
                                ft_s = ft_halves[k // (KT // 2)]
                                fo = (k % (KT // 2)) * 128
                                ps = pp1.tile([128, H], F32, tag="p64", bufs=2)
                                if k == 0:
                                    absorb(ps, 128, H)
                                nc.tensor.matmul(
                                    ps[:], ft_s[:, fo : fo + 128], wn_s[:],
                                    start=True, stop=True,
                                )
                                h0f = ftp.tile([128, H], F32, tag="h0f", bufs=4)
                                nc.scalar.activation(h0f[:], ps[:], RELU)
                                hl = hilopool.tile([128, 128], F16,
                                                   name=f"h0hl{k}", tag="HL")
                                nc.vector.tensor_copy(hl[:, 0:H], h0f[:])
                                nc.vector.tensor_sub(hl[:, H:128], h0f[:],
                                                     hl[:, 0:H])
                                h0_tiles.append(hl)
                            if kk >= LAG:
                                k = kk - LAG
                                if k == 0:
                                    absorb(psa0, 128, 512)
                                nc.tensor.matmul(
                                    psa0[:], h0_tiles[k], a_slice(k, 0),
                                    start=(k == 0), stop=(k == KT - 1),
                                )

                    def round1_tail(n, psa, agi, ago):
                        rd = persist
                        agg16 = rd.tile([128, 512], F16, tag=f"agg{n}")
                        nc.scalar.copy(agg16[:], psa[:])
                        res16 = rd.tile([H, 512], F16, tag=f"res{n}")
                        nc.vector.tensor_sub(res16[:], psa[0:H, :], agg16[0:H, :])
                        psw = prd.tile([H, 512], F32, tag="psw", bufs=1)
                        if n == 0:
                            absorb(psw, H, 512)
                        nc.tensor.matmul(psw[:], wc2h_s[:], agg16[:],
                                         start=True, stop=False)
                        nc.tensor.matmul(psw[:], wc2l_s[:], agg16[:],
                                         start=False, stop=False)
                        nc.tensor.matmul(psw[:], wc1h_s[:], res16[:],
                                         start=False, stop=True)
                        # h1 n-half: fp16 hi only (64 KB over the wire)
                        hiT = rd.tile([H, 512], F16, tag=f"hiT{n}")
                        nc.scalar.activation(hiT[:], psw[:], RELU, bias=bc_s[:])
                        for mm in range(MT // 2):
                            pst = prd.tile([128, H], F16, tag="pst", bufs=2)
                            nc.tensor.transpose(
                                pst[:], hiT[:, mm * 128 : (mm + 1) * 128],
                                ident[:],
                            )
                            nrm = rd.tile([128, H], F16, tag=f"nrm{n}", bufs=4)
                            nc.vector.tensor_copy(nrm[:], pst[:])
                            nc.scalar.dma_start(
                                out=agi[mm * 128 : (mm + 1) * 128, :], in_=nrm[:],
                            )
                        nc.gpsimd.collective_compute(
                            "AllGather", mybir.AluOpType.bypass,
                            replica_groups=rg, ins=[agi[:]], outs=[ago[:]],
                        )

                    round1_tail(0, psa0, ag1a_in, ag1a_out)

                    for k in range(KT):
                        nc.tensor.matmul(
                            psa1[:], h0_tiles[k], a_slice(k, 1),
                            start=(k == 0), stop=(k == KT - 1),
                        )
                    round1_tail(1, psa1, ag1b_in, ag1b_out)
                    warmers(20, psa0)

                # ---- unpack gathered h1 (hi-only) and run round 2 ----------
                cur_tiles = [None] * KT
                korder = []
                for half, ago in [(0, ag1a_out), (1, ag1b_out)]:
                    for g in range(8):
                        hl4 = hilopool.tile(
                            [128, 4 * H], F16, name=f"h1h{half}_{g}",
                            tag="HL4", bufs=16,
                        )
                        src = ago[g * 512 : (g + 1) * 512, :].rearrange(
                            "(t p) c -> p t c", p=128
                        )
                        nc.scalar.dma_start(
                            out=hl4[:].rearrange("p (t c) -> p t c", t=4),
                            in_=src,
                        )
                        for t in range(4):
                            k = g * 8 + half * 4 + t
                            cur_tiles[k] = hl4[:, t * H : (t + 1) * H]
                            korder.append(k)

                with tc.tile_pool(name="prd2", bufs=1, space="PSUM") as prd2:
                    for n in range(2):
                        nsl = slice(n * 512, (n + 1) * 512)
                        psa = prd2.tile([H, 512], F32, tag="psa2", bufs=2)
                        if n == 0:
                            absorb(psa, H, 512)
                        for ki, k in enumerate(korder):
                            nc.tensor.matmul(
                                psa[:], cur_tiles[k], a_slice(k, n),
                                start=(ki == 0), stop=(ki == KT - 1),
                            )
                        agg16 = persist.tile([H, 512], F16, tag=f"agg2_{n}")
                        nc.scalar.copy(agg16[:], psa[:])
                        res16 = persist.tile([H, 512], F16, tag=f"res2_{n}")
                        nc.vector.tensor_sub(res16[:], psa[:], agg16[:])
                        psw = prd2.tile([H, 512], F32, tag="psw2", bufs=2)
                        if n == 0:
                            absorb(psw, H, 512)
                        nc.tensor.matmul(psw[:], wc1h_s[:], agg16[:],
                                         start=True, stop=False)
                        nc.tensor.matmul(psw[:], wc1l_s[:], agg16[:],
                                         start=False, stop=False)
                        nc.tensor.matmul(psw[:], wc1h_s[:], res16[:],
                                         start=False, stop=True)
                        # final h, x0.5 (so sim/4 fits fp16), hi/lo fp16
                        nc.scalar.activation(
                            hThl[0:H, nsl], psw[:], RELU, bias=bch_s[:],
                            scale=0.5,
                        )
                        hi32 = persist.tile([H, 512], F32, tag=f"h2f{n}")
                        nc.scalar.activation(
                            hi32[:], psw[:], RELU, bias=bch_s[:], scale=0.5,
                        )
                        nc.vector.tensor_sub(
                            hThl[H:128, nsl], hi32[:], hThl[0:H, nsl]
                        )
                    nc.scalar.dma_start(out=ag2_in[:], in_=hThl[0:H, :])
                    nc.gpsimd.collective_compute(
                        "AllGather", mybir.AluOpType.bypass,
                        replica_groups=rg, ins=[ag2_in[:]], outs=[ag2_out[:]],
                    )
                    warmers(20, psa)

            # ---------------- phase 3: similarity row-slab ------------------
            # Rotated rhs: local slot s holds rank (c+s)%8, hi rows
            # duplicated onto partitions 64:128 via the index table, so the
            # [hi;lo] stationary pairs with [hi;hi] moving at full PE rate.
            with (
                tc.tile_pool(name="ph3", bufs=1) as ph3,
                tc.tile_pool(name="stg", bufs=6) as stg,
                tc.tile_pool(name="pp3", bufs=1, space="PSUM") as pp3,
            ):
                rhs = ph3.tile([128, N], F16, tag="rhs")
                for s in range(NCORES):
                    nc.gpsimd.indirect_dma_start(
                        out=rhs[:, s * NB : (s + 1) * NB],
                        out_offset=None,
                        in_=ag2_out[:],
                        in_offset=bass.IndirectOffsetOnAxis(
                            ap=ridx_s[:, s : s + 1], axis=0
                        ),
                    )

                first = True
                for m in range(MT):
                    q = m // 4
                    msl = slice(m * 128, (m + 1) * 128)
                    units = [(0, 2), (2, 2), (4, 2), (6, 2), (8, 1)]
                    for ui, (t0, w) in enumerate(units):
                        ps3 = pp3.tile([128, w * 512], F32, tag=f"ps{w}",
                                       bufs=(3 if w == 2 else 2))
                        if first:
                            absorb(ps3[:, 0:512], 128, 512)
                            first = False
                        for dt_ in range(w):
                            u = q + t0 + dt_
                            nc.tensor.matmul(
                                ps3[:, dt_ * 512 : (dt_ + 1) * 512],
                                hThl[:, msl],
                                rhs[:, u * 512 : (u + 1) * 512],
                                start=True, stop=True,
                            )
                        st = stg.tile([128, w * 512], F16, tag=f"st{w}",
                                      bufs=(6 if w == 2 else 3))
                        if (m * 5 + ui) % 2 == 0:
                            nc.scalar.copy(st[:], ps3[:])
                        else:
                            nc.vector.tensor_copy(st[:], ps3[:])
                        dst0 = (q + t0) * 512
                        nc.sync.dma_start(
                            out=out_ext[msl, dst0 : dst0 + w * 512], in_=st[:]
                        )
    _legalize_waits(nc)
    return nc


def _host_prep(features, W_node, b_node, W_conv, b_conv, nodes, edges):
    features = np.asarray(features, np.float32)
    W_node = np.asarray(W_node, np.float32)
    b_node = np.asarray(b_node, np.float32)
    W_conv = np.asarray(W_conv, np.float32)
    b_conv = np.asarray(b_conv, np.float32)
    edges = np.asarray(edges)

    def _hilo_bf(x):
        hi = x.astype(ml_dtypes.bfloat16)
        lo = (x - hi.astype(np.float32)).astype(ml_dtypes.bfloat16)
        return hi, lo

    fa = np.concatenate([features.T, np.ones((1, N), np.float32)], axis=0)
    Wa = np.concatenate([W_node, b_node[None, :]], axis=0)
    fa_hi, fa_lo = _hilo_bf(fa)
    fa_lo_z = fa_lo.copy()
    fa_lo_z[F, :] = 0  # no double-counted bias
    Wa_hi, Wa_lo = _hilo_bf(Wa)
    featT3 = np.concatenate([fa_hi, fa_lo_z, fa_hi], axis=0)  # [33, N] bf16
    W3 = np.concatenate([Wa_hi, Wa_hi, Wa_lo], axis=0)  # [33, H] bf16

    Wc_hi = W_conv.astype(np.float16)
    Wc_lo = (W_conv - Wc_hi.astype(np.float32)).astype(np.float16)
    Wc2h = np.concatenate([Wc_hi, Wc_hi], axis=0)  # [128, H] fp16
    Wc2l = np.concatenate([Wc_lo, Wc_lo], axis=0)
    bcv = b_conv.reshape(H, 1)
    bch = (0.5 * b_conv).reshape(H, 1)

    src = edges[:, 0].astype(np.int64)
    dst = edges[:, 1].astype(np.int64)
    in_maps = []
    for c in range(NCORES):
        sel = (dst >= c * NB) & (dst < (c + 1) * NB)
        idx = src[sel] * NB + (dst[sel] - c * NB)
        cnt = np.bincount(idx, minlength=N * NB).astype(np.float32).reshape(N, NB)
        cnt[c * NB + np.arange(NB), np.arange(NB)] += 1.0  # fold identity
        assert cnt.max() <= 16, "adjacency counts exceed exact fp8 range"
        atp = np.ascontiguousarray(
            cnt.reshape(KT // 2, 2, 128, NB).transpose(2, 0, 1, 3).reshape(128, KT * NB)
        )
        p = np.arange(128)
        s = np.arange(NCORES)
        ridx = (64 * ((c + s[None, :]) % NCORES) + (p[:, None] % 64)).astype(
            np.int32
        )
        in_maps.append(
            {
                "featT3": featT3,
                "W3": W3,
                "Wc2h": Wc2h,
                "Wc2l": Wc2l,
                "Wc1h": Wc_hi,
                "Wc1l": Wc_lo,
                "bc": bcv,
                "bch": bch,
                "ridx": ridx,
                "AT": atp.astype(ml_dtypes.float8_e4m3),
            }
        )
    return in_maps


def kernel(features, W_node, b_node, W_conv, b_conv, nodes, edges, **kw):
    global LAST_RESULT
    _ensure_trace_hook()
    in_maps = _host_prep(features, W_node, b_node, W_conv, b_conv, nodes, edges)
    nc = _build_nc()
    res = run_bass_kernel_spmd(nc, in_maps, core_ids=list(range(NCORES)))
    LAST_RESULT = res
    out = np.empty((2, N, N), np.float32)
    sim = out[1]
    # direct writes: row-512-group P = 2c+q owns col groups (P+t)%16, t<=8
    for c in range(NCORES):
        dev = res.results[c]["out"]  # [NB, 10*512] fp16, x1/4
        for q in range(2):
            slab = dev[512 * q : 512 * (q + 1),
                       512 * q : 512 * q + NW * 512].astype(np.float32)
            slab *= 4.0
            r0 = 1024 * c + 512 * q
            for t in range(NW):
                G = (2 * c + q + t) % 16
                sim[r0 : r0 + 512, 512 * G : 512 * G + 512] = \
                    slab[:, 512 * t : 512 * (t + 1)]
    # mirror the remaining blocks (cyclic distance 9..15)
    for P in range(16):
        for d in range(NW, 16):
            G = (P + d) % 16
            sim[512 * P : 512 * P + 512, 512 * G : 512 * G + 512] = \
                sim[512 * G : 512 * G + 512, 512 * P : 512 * P + 512].T
    # function_deps is similarity with rows/cols masked to nodes == 2
    out[0] = 0.0
    idx = np.flatnonzero(np.asarray(nodes) == 2)
    ix = np.ix_(idx, idx)
    out[0][ix] = out[1][ix]
    return out


if __name__ == "__main__":
    np.random.seed(0)
    feats = np.random.randn(N, F).astype(np.float32)
    ins = {
        "features": feats,
        "W_node": (np.random.randn(F, H) * 0.1).astype(np.float32),
        "b_node": (np.random.randn(H) * 0.1).astype(np.float32),
        "W_conv": (np.random.randn(H, H) * 0.05).astype(np.float32),
        "b_conv": (np.random.randn(H) * 0.05).astype(np.float32),
        "nodes": np.random.randint(0, 5, N, dtype=np.int32),
        "edges": np.random.randint(0, N, (524288, 2), dtype=np.int32),
    }
    out = kernel(**ins)
    print(out.shape, out.dtype)


# revision 9
# speedup vs baseline: 1.2936x; 1.2936x over previous
"""Trainium2 Bass kernel for the DependencyAnalyzer GNN problem.

Computation (reference semantics):
    h = relu(features @ W_node + b_node)                  # [N, H]
    2x: agg = scatter_add(h[src] -> dst);  h = relu((h + agg) @ W_conv + b_conv)
    out = stack([ (m*h) @ (m*h).T,  h @ h.T ])            # m = (nodes == 2)

Strategy (8 NeuronCores, SPMD):
  - Host reformats the edge list into per-core dense adjacency blocks
    A'^T [src=8192, dst_local=1024] in fp8 (counts <= 16 are exact), with
    the identity folded in (A' = A + I); A' stays fp8 end-to-end as the
    *moving* matmul operand (full rate against an fp16 stationary side).
  - h0 (replicated) is software-pipelined into the round-1 n=0
    aggregation loop so the first AllGather input is ready right behind
    the A-load stream.
  - AllGathers carry fp16 hi-only h (64-128 KB): the collective runs at
    the fold_n bandwidth ceiling, so payload is wall-clock.
  - similarity row-slabs exploit symmetry: each 512-row group writes
    only col-groups at cyclic distance 0..8; the host reconstructs the
    rest by transposing mirror blocks.  The needed per-core rotation of
    the gathered h2 (local col-slot s <- rank (c+s)%8) is done with
    indirect DMA gathers driven by a host-provided index table — the
    only per-core-variant addressing in an otherwise shared program.
  - On-device output is fp16 scaled by 1/4 (exact rescale on host);
    function_deps is a row/col-masked copy of similarity applied on the
    host during unsharding.
"""

import numpy as np
import ml_dtypes

import concourse.bass as bass
import concourse.mybir as mybir
import concourse.tile as tile
from concourse import masks
from concourse.bass_utils import run_bass_kernel_spmd

N = 8192          # nodes
NB = 1024         # nodes per core block
NCORES = 8
F = 10            # feature dim
FA = F + 1        # +1 ones row (bias fold)
H = 64            # hidden dim
KT = N // 128     # 64 src k-tiles
MT = NB // 128    # 8 own m-tiles
AC = 8            # A-load chunks (8 k-tiles each)
NW = 9            # owned col-512-tiles per 512-row group (cyclic window)
F32 = mybir.dt.float32
BF16 = mybir.dt.bfloat16
F16 = mybir.dt.float16
F8 = mybir.dt.float8e4
I32 = mybir.dt.int32
RELU = mybir.ActivationFunctionType.Relu

LAST_RESULT = None  # BassKernelResults of the most recent run (for test harness)


def _ensure_trace_hook():
    """Best-effort: register the NTFF profiling hook for trace=True runs."""
    import sys as _sys
    import types as _types

    try:
        if "antenv.axon_hooks" in _sys.modules:
            return
        import antenv as _antenv

        mod = _types.ModuleType("antenv.axon_hooks")
        _state = {"hook": None}
        mod.set_axon_ntff_profile_hook = lambda h: _state.__setitem__("hook", h)
        mod.get_axon_ntff_profile_hook = lambda: _state["hook"]
        _sys.modules["antenv.axon_hooks"] = mod
        _antenv.axon_hooks = mod

        from trn_agent_boot.trn_boot import _ntff_profile_via_ctypes

        so_path = "/opt/axon/libaxon_pjrt.so"
        import os as _os

        if _os.path.exists(so_path):
            hook = _ntff_profile_via_ctypes(so_path)
            if hook is not None:
                mod.set_axon_ntff_profile_hook(hook)
    except Exception:
        pass


def _legalize_waits(nc, max_waits=1):
    """This walrus build accepts at most one sync-wait per lowered HW
    instruction; hoist extra waits onto standalone EventSemaphore
    instructions on the same (in-order) engine queue."""
    n_fixed = 0
    for f in nc.m.functions:
        for bb in f.blocks:
            new_list = []
            for ins in bb.instructions:
                si = ins.sync_info
                if si is not None and len(si.on_wait) > max_waits:
                    waits = list(si.on_wait)
                    for w in waits[: len(waits) - max_waits]:
                        ev = mybir.InstEventSemaphore(
                            name=f"{ins.name}-w-{w.ant_name}",
                            ins=[],
                            outs=[],
                            sync_info=mybir.SyncInfo(on_wait=[w], on_update=[]),
                            engine=ins.engine,
                        )
                        new_list.append(ev)
                    ins.sync_info = mybir.SyncInfo(
                        on_wait=waits[len(waits) - max_waits :],
                        on_update=list(si.on_update),
                    )
                    n_fixed += 1
                new_list.append(ins)
            bb.instructions = new_list
    return n_fixed


def _build_nc():
    nc = bass.Bass(num_devices=NCORES)

    # ---- external I/O (same program on all cores; per-core data differs) ----
    featT = nc.declare_dram_parameter("featT3", [3 * FA, N], BF16, isOutput=False)
    WnA = nc.declare_dram_parameter("W3", [3 * FA, H], BF16, isOutput=False)
    Wc2h = nc.declare_dram_parameter("Wc2h", [2 * H, H], F16, isOutput=False)
    Wc2l = nc.declare_dram_parameter("Wc2l", [2 * H, H], F16, isOutput=False)
    Wc1h = nc.declare_dram_parameter("Wc1h", [H, H], F16, isOutput=False)
    Wc1l = nc.declare_dram_parameter("Wc1l", [H, H], F16, isOutput=False)
    bc = nc.declare_dram_parameter("bc", [H, 1], F32, isOutput=False)
    bch = nc.declare_dram_parameter("bch", [H, 1], F32, isOutput=False)
    # rotation index table: ridx[p, s] = 64*((c+s)%8) + p
    ridx = nc.declare_dram_parameter("ridx", [H, NCORES], I32, isOutput=False)
    # A'^T src-tile-packed: AT[p, k*NB + d] = count for src (k//2)*256 +
    # (k%2)*128 + p, local dst d.
    AT = nc.declare_dram_parameter("AT", [128, KT * NB], F8, isOutput=False)
    out_ext = nc.declare_dram_parameter("out", [NB, (NW + 1) * 512], F16,
                                        isOutput=True)

    # ---- internal DRAM (collective bounce buffers) ----
    ag1a_in = nc.dram_tensor("ag1a_in", [NB // 2, H], F16)
    ag1a_out = nc.dram_tensor("ag1a_out", [N // 2, H], F16, addr_space="Shared")
    ag1b_in = nc.dram_tensor("ag1b_in", [NB // 2, H], F16)
    ag1b_out = nc.dram_tensor("ag1b_out", [N // 2, H], F16, addr_space="Shared")
    agd_in = nc.dram_tensor("agd_in", [1, H], F16)
    agd_out = nc.dram_tensor("agd_out", [NCORES, H], F16, addr_space="Shared")
    ag2_in = nc.dram_tensor("ag2_in", [H, NB], F16)
    ag2_out = nc.dram_tensor("ag2_out", [NCORES * H, NB], F16, addr_space="Shared")
    rg = [list(range(NCORES))]

    with tile.TileContext(nc, num_cores=NCORES) as tc:
        with tc.tile_pool(name="persist", bufs=1) as persist:
            # ---------------- constants / small inputs -----------------------
            # wn/bc on ACT HWDGE (needed early); other consts via SWDGE;
            # ft + A stream + outputs on the SP HWDGE queue.
            wn_s = persist.tile([3 * FA, H], BF16)
            nc.scalar.dma_start(out=wn_s[:], in_=WnA[:])
            bc_s = persist.tile([H, 1], F32)
            nc.scalar.dma_start(out=bc_s[:], in_=bc[:])
            bch_s = persist.tile([H, 1], F32)
            nc.scalar.dma_start(out=bch_s[:], in_=bch[:])
            ridx_s = persist.tile([H, NCORES], I32)
            nc.scalar.dma_start(out=ridx_s[:], in_=ridx[:])
            # tiny AllGather issued immediately: it absorbs the cc-stream
            # bootstrap + first-op control latency and rank skew, so the
            # first real gather runs on a hot stream.
            agd_t = persist.tile([1, H], F16)
            nc.vector.memset(agd_t[:], 0.0)
            nc.scalar.dma_start(out=agd_in[:], in_=agd_t[:])
            nc.gpsimd.collective_compute(
                "AllGather", mybir.AluOpType.bypass,
                replica_groups=rg, ins=[agd_in[:]], outs=[agd_out[:]],
            )
            wc2h_s = persist.tile([2 * H, H], F16)
            nc.gpsimd.dma_start(out=wc2h_s[:], in_=Wc2h[:])
            wc2l_s = persist.tile([2 * H, H], F16)
            nc.gpsimd.dma_start(out=wc2l_s[:], in_=Wc2l[:])
            wc1h_s = persist.tile([H, H], F16)
            nc.gpsimd.dma_start(out=wc1h_s[:], in_=Wc1h[:])
            wc1l_s = persist.tile([H, H], F16)
            nc.gpsimd.dma_start(out=wc1l_s[:], in_=Wc1l[:])
            ident = persist.tile([H, H], F16)
            masks.make_identity(nc, ident[:])
            dummy_s = persist.tile([1, 512], BF16)
            nc.vector.memset(dummy_s[:], 0.0)

            def absorb(pt, parts, free):
                # Dummy full-tile matmul: soaks up PSUM pool-boundary WAR
                # waits on PE so real matmuls stay within the ISA's sync
                # wait budget.
                nc.tensor.matmul(
                    pt[:, :],
                    dummy_s[0:1, 0:parts],
                    dummy_s[0:1, 0:free],
                    start=True,
                    stop=True,
                )

            def warmers(n_fill, fill_ps):
                # Dependency-free matmuls into an already-drained psum bank;
                # they run while the PE would otherwise idle in a collective
                # wait, keeping the HAM clock gate at full rate.
                for _ in range(n_fill):
                    nc.tensor.matmul(
                        fill_ps[0:1, :], dummy_s[0:1, 0:1], dummy_s[0:1, :],
                        start=True, stop=True,
                    )

            # final-h (hi/lo fp16, T layout, x0.5) for the output matmuls
            hThl = persist.tile([128, NB], F16)

            with (
                tc.tile_pool(name="apool", bufs=AC) as apool,
                tc.tile_pool(name="hilo", bufs=KT) as hilopool,
                tc.tile_pool(name="ftp", bufs=2) as ftp,
            ):
                ft_halves = []
                for half in range(2):
                    ft_h = ftp.tile([3 * FA, N // 2], BF16, tag=f"ft{half}", bufs=1)
                    nc.sync.dma_start(
                        out=ft_h[:],
                        in_=featT[:, half * (N // 2) : (half + 1) * (N // 2)],
                    )
                    ft_halves.append(ft_h)

                a_chunks = []
                for j in range(AC):
                    at = apool.tile([128, (KT // AC) * NB], F8, name=f"a{j}",
                                    tag="A")
                    nc.sync.dma_start(
                        out=at[:],
                        in_=AT[:, j * (KT // AC) * NB : (j + 1) * (KT // AC) * NB],
                    )
                    a_chunks.append(at)

                def a_slice(k, n):
                    at = a_chunks[k // (KT // AC)]
                    off = (k % (KT // AC)) * NB + n * 512
                    return at[:, off : off + 512]

                # ---- phase 1 + round-1 n=0, software-pipelined -------------
                with tc.tile_pool(name="prd1", bufs=1, space="PSUM") as prd:
                    psa0 = prd.tile([128, 512], F32, tag="psa", bufs=2)
                    psa1 = prd.tile([128, 512], F32, tag="psa", bufs=2)
                    h0_tiles = []
                    LAG = 3
                    with tc.tile_pool(name="pp1", bufs=2, space="PSUM") as pp1:
                        for kk in range(KT + LAG):
                            if kk < KT:
                                k = kk
                                ft_s = ft_halves[k // (KT // 2)]
                                fo = (k % (KT // 2)) * 128
                                ps = pp1.tile([128, H], F32, tag="p64", bufs=2)
                                if k == 0:
                                    absorb(ps, 128, H)
                                nc.tensor.matmul(
                                    ps[:], ft_s[:, fo : fo + 128], wn_s[:],
                                    start=True, stop=True,
                                )
                                h0f = ftp.tile([128, H], F32, tag="h0f", bufs=4)
                                nc.scalar.activation(h0f[:], ps[:], RELU)
                                hl = hilopool.tile([128, 128], F16,
                                                   name=f"h0hl{k}", tag="HL")
                                nc.vector.tensor_copy(hl[:, 0:H], h0f[:])
                                nc.vector.tensor_sub(hl[:, H:128], h0f[:],
                                                     hl[:, 0:H])
                                h0_tiles.append(hl)
                            if kk >= LAG:
                                k = kk - LAG
                                if k == 0:
                                    absorb(psa0, 128, 512)
                                nc.tensor.matmul(
                                    psa0[:], h0_tiles[k], a_slice(k, 0),
                                    start=(k == 0), stop=(k == KT - 1),
                                )

                    def round1_tail(n, psa, agi, ago):
                        rd = persist
                        agg16 = rd.tile([128, 512], F16, tag=f"agg{n}")
                        nc.scalar.copy(agg16[:], psa[:])
                        res16 = rd.tile([H, 512], F16, tag=f"res{n}")
                        nc.vector.tensor_sub(res16[:], psa[0:H, :], agg16[0:H, :])
                        psw = prd.tile([H, 512], F32, tag="psw", bufs=1)
                        if n == 0:
                            absorb(psw, H, 512)
                        nc.tensor.matmul(psw[:], wc2h_s[:], agg16[:],
                                         start=True, stop=False)
                        nc.tensor.matmul(psw[:], wc2l_s[:], agg16[:],
                                         start=False, stop=False)
                        nc.tensor.matmul(psw[:], wc1h_s[:], res16[:],
                                         start=False, stop=True)
                        # h1 n-half: fp16 hi only (64 KB over the wire)
                        hiT = rd.tile([H, 512], F16, tag=f"hiT{n}")
                        nc.scalar.activation(hiT[:], psw[:], RELU, bias=bc_s[:])
                        for mm in range(MT // 2):
                            pst = prd.tile([128, H], F16, tag="pst", bufs=2)
                            nc.tensor.transpose(
                                pst[:], hiT[:, mm * 128 : (mm + 1) * 128],
                                ident[:],
                            )
                            nrm = rd.tile([128, H], F16, tag=f"nrm{n}", bufs=4)
                            nc.vector.tensor_copy(nrm[:], pst[:])
                            nc.scalar.dma_start(
                                out=agi[mm * 128 : (mm + 1) * 128, :], in_=nrm[:],
                            )
                        nc.gpsimd.collective_compute(
                            "AllGather", mybir.AluOpType.bypass,
                            replica_groups=rg, ins=[agi[:]], outs=[ago[:]],
                        )

                    round1_tail(0, psa0, ag1a_in, ag1a_out)

                    for k in range(KT):
                        nc.tensor.matmul(
                            psa1[:], h0_tiles[k], a_slice(k, 1),
                            start=(k == 0), stop=(k == KT - 1),
                        )
                    round1_tail(1, psa1, ag1b_in, ag1b_out)
                    warmers(20, psa0)

                # ---- unpack gathered h1 (hi-only) and run round 2 ----------
                cur_tiles = [None] * KT
                korder = []
                for half, ago in [(0, ag1a_out), (1, ag1b_out)]:
                    for g in range(8):
                        hl4 = hilopool.tile(
                            [128, 4 * H], F16, name=f"h1h{half}_{g}",
                            tag="HL4", bufs=16,
                        )
                        src = ago[g * 512 : (g + 1) * 512, :].rearrange(
                            "(t p) c -> p t c", p=128
                        )
                        nc.scalar.dma_start(
                            out=hl4[:].rearrange("p (t c) -> p t c", t=4),
                            in_=src,
                        )
                        for t in range(4):
                            k = g * 8 + half * 4 + t
                            cur_tiles[k] = hl4[:, t * H : (t + 1) * H]
                            korder.append(k)

                with tc.tile_pool(name="prd2", bufs=1, space="PSUM") as prd2:
                    for n in range(2):
                        nsl = slice(n * 512, (n + 1) * 512)
                        psa = prd2.tile([H, 512], F32, tag="psa2", bufs=2)
                        if n == 0:
                            absorb(psa, H, 512)
                        for ki, k in enumerate(korder):
                            nc.tensor.matmul(
                                psa[:], cur_tiles[k], a_slice(k, n),
                                start=(ki == 0), stop=(ki == KT - 1),
                            )
                        agg16 = persist.tile([H, 512], F16, tag=f"agg2_{n}")
                        nc.scalar.copy(agg16[:], psa[:])
                        res16 = persist.tile([H, 512], F16, tag=f"res2_{n}")
                        nc.vector.tensor_sub(res16[:], psa[:], agg16[:])
                        psw = prd2.tile([H, 512], F32, tag="psw2", bufs=2)
                        if n == 0:
                            absorb(psw, H, 512)
                        nc.tensor.matmul(psw[:], wc1h_s[:], agg16[:],
                                         start=True, stop=False)
                        nc.tensor.matmul(psw[:], wc1l_s[:], agg16[:],
                                         start=False, stop=False)
                        nc.tensor.matmul(psw[:], wc1h_s[:], res16[:],
                                         start=False, stop=True)
                        # final h, x0.5 (so sim/4 fits fp16), hi/lo fp16
                        nc.scalar.activation(
                            hThl[0:H, nsl], psw[:], RELU, bias=bch_s[:],
                            scale=0.5,
                        )
                        hi32 = persist.tile([H, 512], F32, tag=f"h2f{n}")
                        nc.scalar.activation(
                            hi32[:], psw[:], RELU, bias=bch_s[:], scale=0.5,
                        )
                        nc.vector.tensor_sub(
                            hThl[H:128, nsl], hi32[:], hThl[0:H, nsl]
                        )
                    nc.scalar.dma_start(out=ag2_in[:], in_=hThl[0:H, :])
                    nc.gpsimd.collective_compute(
                        "AllGather", mybir.AluOpType.bypass,
                        replica_groups=rg, ins=[ag2_in[:]], outs=[ag2_out[:]],
                    )
                    warmers(20, psa)

            # ---------------- phase 3: similarity row-slab ------------------
            # Rotated rhs: local slot s holds rank (c+s)%8, hi rows
            # duplicated onto partitions 64:128 via the index table, so the
            # [hi;lo] stationary pairs with [hi;hi] moving at full PE rate.
            with (
                tc.tile_pool(name="ph3", bufs=1) as ph3,
                tc.tile_pool(name="stg", bufs=6) as stg,
                tc.tile_pool(name="pp3", bufs=1, space="PSUM") as pp3,
            ):
                rhs = ph3.tile([128, N], F16, tag="rhs")
                for s in range(NCORES):
                    ssl = slice(s * NB, (s + 1) * NB)
                    nc.gpsimd.indirect_dma_start(
                        out=rhs[0:H, ssl],
                        out_offset=None,
                        in_=ag2_out[:],
                        in_offset=bass.IndirectOffsetOnAxis(
                            ap=ridx_s[:, s : s + 1], axis=0
                        ),
                    )
                    nc.sync.dma_start(out=rhs[H:128, ssl], in_=rhs[0:H, ssl])

                first = True
                units = [(0, 2), (2, 2), (4, 2), (6, 2), (8, 1)]
                for ui, (t0, w) in enumerate(units):
                    for m in range(MT):
                        q = m // 4
                        msl = slice(m * 128, (m + 1) * 128)
                        ps3 = pp3.tile([128, w * 512], F32, tag=f"ps{w}",
                                       bufs=(3 if w == 2 else 2))
                        if first:
                            absorb(ps3[:, 0:512], 128, 512)
                            first = False
                        for dt_ in range(w):
                            u = q + t0 + dt_
                            nc.tensor.matmul(
                                ps3[:, dt_ * 512 : (dt_ + 1) * 512],
                                hThl[:, msl],
                                rhs[:, u * 512 : (u + 1) * 512],
                                start=True, stop=True,
                            )
                        st = stg.tile([128, w * 512], F16, tag=f"st{w}",
                                      bufs=(6 if w == 2 else 3))
                        if (m * 5 + ui) % 2 == 0:
                            nc.scalar.copy(st[:], ps3[:])
                        else:
                            nc.vector.tensor_copy(st[:], ps3[:])
                        dst0 = (q + t0) * 512
                        nc.sync.dma_start(
                            out=out_ext[msl, dst0 : dst0 + w * 512], in_=st[:]
                        )
    _legalize_waits(nc)
    return nc


def _host_prep(features, W_node, b_node, W_conv, b_conv, nodes, edges):
    features = np.asarray(features, np.float32)
    W_node = np.asarray(W_node, np.float32)
    b_node = np.asarray(b_node, np.float32)
    W_conv = np.asarray(W_conv, np.float32)
    b_conv = np.asarray(b_conv, np.float32)
    edges = np.asarray(edges)

    def _hilo_bf(x):
        hi = x.astype(ml_dtypes.bfloat16)
        lo = (x - hi.astype(np.float32)).astype(ml_dtypes.bfloat16)
        return hi, lo

    fa = np.concatenate([features.T, np.ones((1, N), np.float32)], axis=0)
    Wa = np.concatenate([W_node, b_node[None, :]], axis=0)
    fa_hi, fa_lo = _hilo_bf(fa)
    fa_lo_z = fa_lo.copy()
    fa_lo_z[F, :] = 0  # no double-counted bias
    Wa_hi, Wa_lo = _hilo_bf(Wa)
    featT3 = np.concatenate([fa_hi, fa_lo_z, fa_hi], axis=0)  # [33, N] bf16
    W3 = np.concatenate([Wa_hi, Wa_hi, Wa_lo], axis=0)  # [33, H] bf16

    Wc_hi = W_conv.astype(np.float16)
    Wc_lo = (W_conv - Wc_hi.astype(np.float32)).astype(np.float16)
    Wc2h = np.concatenate([Wc_hi, Wc_hi], axis=0)  # [128, H] fp16
    Wc2l = np.concatenate([Wc_lo, Wc_lo], axis=0)
    bcv = b_conv.reshape(H, 1)
    bch = (0.5 * b_conv).reshape(H, 1)

    src = edges[:, 0].astype(np.int64)
    dst = edges[:, 1].astype(np.int64)
    in_maps = []
    for c in range(NCORES):
        sel = (dst >= c * NB) & (dst < (c + 1) * NB)
        idx = src[sel] * NB + (dst[sel] - c * NB)
        cnt = np.bincount(idx, minlength=N * NB).astype(np.float32).reshape(N, NB)
        cnt[c * NB + np.arange(NB), np.arange(NB)] += 1.0  # fold identity
        assert cnt.max() <= 16, "adjacency counts exceed exact fp8 range"
        atp = np.ascontiguousarray(
            cnt.reshape(KT // 2, 2, 128, NB).transpose(2, 0, 1, 3).reshape(128, KT * NB)
        )
        p = np.arange(H)
        s = np.arange(NCORES)
        ridx = (64 * ((c + s[None, :]) % NCORES) + p[:, None]).astype(np.int32)
        in_maps.append(
            {
                "featT3": featT3,
                "W3": W3,
                "Wc2h": Wc2h,
                "Wc2l": Wc2l,
                "Wc1h": Wc_hi,
                "Wc1l": Wc_lo,
                "bc": bcv,
                "bch": bch,
                "ridx": ridx,
                "AT": atp.astype(ml_dtypes.float8_e4m3),
            }
        )
    return in_maps


def kernel(features, W_node, b_node, W_conv, b_conv, nodes, edges, **kw):
    global LAST_RESULT
    _ensure_trace_hook()
    in_maps = _host_prep(features, W_node, b_node, W_conv, b_conv, nodes, edges)
    nc = _build_nc()
    res = run_bass_kernel_spmd(nc, in_maps, core_ids=list(range(NCORES)))
    LAST_RESULT = res
    out = np.empty((2, N, N), np.float32)
    sim = out[1]
    # direct writes: row-512-group P = 2c+q owns col groups (P+t)%16, t<=8
    for c in range(NCORES):
        dev = res.results[c]["out"]  # [NB, 10*512] fp16, x1/4
        for q in range(2):
            slab = dev[512 * q : 512 * (q + 1),
                       512 * q : 512 * q + NW * 512].astype(np.float32)
            slab *= 4.0
            r0 = 1024 * c + 512 * q
            for t in range(NW):
                G = (2 * c + q + t) % 16
                sim[r0 : r0 + 512, 512 * G : 512 * G + 512] = \
                    slab[:, 512 * t : 512 * (t + 1)]
    # mirror the remaining blocks (cyclic distance 9..15)
    for P in range(16):
        for d in range(NW, 16):
            G = (P + d) % 16
            sim[512 * P : 512 * P + 512, 512 * G : 512 * G + 512] = \
                sim[512 * G : 512 * G + 512, 512 * P : 512 * P + 512].T
    # function_deps is similarity with rows/cols masked to nodes == 2
    out[0] = 0.0
    idx = np.flatnonzero(np.asarray(nodes) == 2)
    ix = np.ix_(idx, idx)
    out[0][ix] = out[1][ix]
    return out


if __name__ == "__main__":
    np.random.seed(0)
    feats = np.random.randn(N, F).astype(np.float32)
    ins = {
        "features": feats,
        "W_node": (np.random.randn(F, H) * 0.1).astype(np.float32),
        "b_node": (np.random.randn(H) * 0.1).astype(np.float32),
        "W_conv": (np.random.randn(H, H) * 0.05).astype(np.float32),
        "b_conv": (np.random.randn(H) * 0.05).astype(np.float32),
        "nodes": np.random.randint(0, 5, N, dtype=np.int32),
        "edges": np.random.randint(0, N, (524288, 2), dtype=np.int32),
    }
    out = kernel(**ins)
    print(out.shape, out.dtype)
